# revision 14
# baseline (speedup 1.0000x reference)
"""BCQLinear packed forward on 8 Trainium2 NeuronCores.

Column-parallel (tensor-parallel) sharding: binary/alpha/bias are sharded
along out_features (dim 0, 4096 -> 8 x 512); the input activations are
replicated. Each core dequantizes its weight shard
    W[o, i] = sum_b alpha[o, g, b] * binary[o, g, a, b],   i = 128 g + a
on the vector engine (gpsimd-assisted) in bf16, transposes each 128x128
group block through the PE array (one bf16 transpose per group), and runs
W-stationary bf16 matmuls  out[o, t] = Wt[:, o]^T @ x[:, t]  over token
blocks. Cells (o-tile, token-block) are emitted in DMA/dequant arrival
order so the PE never starves during the fill phase; bias is added on the
Activation engine during the PSUM->SBUF copy (per-partition bias, since
out is [o_p, t]). The host concatenates the 8 output shards along o.

Shapes are hardcoded for the problem instance:
  input  [2, 1024, 4096] f32
  binary [4096, 32, 128, 3] f32 (+-1)
  alpha  [4096, 32, 3] f32
  bias   [4096] f32
"""

import numpy as np
from contextlib import ExitStack

import bass_rust
import concourse.bass as bass
import concourse.mybir as mybir
import concourse.tile as tile
from concourse.bass_utils import run_bass_kernel_spmd
from concourse.masks import make_identity


def _legalize_waits(nc, max_waits=1):
    """Walrus codegen allows only one sync-wait on (at least) DVE
    TensorTensor instructions. Move excess waits onto injected same-engine
    NoOps placed immediately before the instruction (program order per
    engine preserves the semantics)."""
    seq = 0
    for fn in nc.m.functions:
        for blk in fn.blocks:
            new_insts = []
            changed = False
            for inst in blk.instructions:
                si = inst.sync_info
                if si is not None and len(si.on_wait) > max_waits:
                    waits = list(si.on_wait)
                    for w in waits[:-max_waits]:
                        nop = mybir.InstNoOp(name=f"wlegal-{seq}")
                        seq += 1
                        nop.engine = inst.engine
                        nop.sync_info = bass_rust.SyncInfo(
                            on_wait=[w], on_update=[])
                        new_insts.append(nop)
                    inst.sync_info = bass_rust.SyncInfo(
                        on_wait=waits[-max_waits:],
                        on_update=list(si.on_update))
                    changed = True
                new_insts.append(inst)
            if changed:
                blk.instructions = new_insts

P = 128          # partitions
N_CORES = 8
B, S = 2, 1024
MS = B * S       # 2048 tokens
I = 4096         # in_features
O = 4096         # out_features
O_SH = O // N_CORES  # 512 per core
G, A, NB = 32, 128, 3
KT = I // P      # 32 contraction tiles (== G since A == P)
OT = O_SH // P   # 4 o-tiles per core

F32 = mybir.dt.float32
BF16 = mybir.dt.bfloat16
FP8 = mybir.dt.float8e4

_CACHED = {}


def build_nc(gh: int = 8, x_split: int = 2, tb: int = 8, pool_assist: int = 1,
             dummies: int = 30, out_bf16: int = 1, zigzag: int = 1,
             x_av_cal=None, d_av_cal=None) -> bass.Bass:
    TB = tb
    TS = MS // TB
    ODT = BF16 if out_bf16 else F32
    NCH = G // gh            # dequant chunks per o-tile

    nc = bass.Bass("TRN2", target_bir_lowering=False, debug=False)

    # Host-staged layouts (pure relayouts / dtype casts of sharded inputs):
    #  xt    [TB, P, KT, TS] bf16 : xt[tb,p,k,t] = x[tb*TS+t, k*128+p]
    #  bperm [O_SH, NB, G, A] fp8 : binary shard, bit axis outward (+-1 exact)
    #  alpha [O_SH, G, NB] f32
    #  biassh[OT, P] f32          : bias shard split into o-tiles
    xt_d = nc.dram_tensor("xt", [TB, P, KT, TS], BF16, kind="ExternalInput").ap()
    b_d = nc.dram_tensor("bperm", [O_SH, NB, G, A], FP8, kind="ExternalInput").ap()
    al_d = nc.dram_tensor("alpha", [O_SH, G, NB], F32, kind="ExternalInput").ap()
    bias_d = nc.dram_tensor("biassh", [OT, P], F32, kind="ExternalInput").ap()
    out_d = nc.dram_tensor("out", [OT, P, MS], ODT, kind="ExternalOutput").ap()

    mult = mybir.AluOpType.mult
    add = mybir.AluOpType.add

    # instruction-name maps for the calibration harness (tune.py)
    nc._x_dma = {t: [] for t in range(TB)}
    nc._deq = {ot: [] for ot in range(OT)}

    with tile.TileContext(nc) as tc, ExitStack() as ctx:
        const = ctx.enter_context(tc.tile_pool(name="const", bufs=1))
        xpool = ctx.enter_context(tc.tile_pool(name="x", bufs=TB))
        wtpool = ctx.enter_context(tc.tile_pool(name="wt", bufs=4))
        bpool = ctx.enter_context(tc.tile_pool(name="bin", bufs=5))
        wpool = ctx.enter_context(tc.tile_pool(name="w", bufs=2))
        opool = ctx.enter_context(tc.tile_pool(name="o", bufs=4))
        ps_mm = ctx.enter_context(tc.tile_pool(name="psmm", bufs=3, space="PSUM"))
        ps_tr = ctx.enter_context(tc.tile_pool(name="pstr", bufs=4, space="PSUM"))

        identb = const.tile([P, P], BF16)
        make_identity(nc, identb)
        al_sb = const.tile([P, OT, G * NB], F32)
        bias_sb = const.tile([P, OT], F32)
        al3 = al_sb.rearrange("p ot (g nb) -> p ot g nb", nb=NB)

        # PE p-state warm-up / gap filler: junk transposes with no data
        # deps keep the PE continuously busy (the cost model restarts the
        # clock ramp at 0.65/1.2 GHz after every idle gap).
        ps_junk = ps_tr.tile([P, P], BF16, tag="junk", bufs=1)

        def emit_junk(us):
            for _ in range(min(400, int(us / 0.0533))):
                nc.tensor.matmul(ps_junk, identb, identb, is_transpose=True,
                                 start=True, stop=True)

        x_tiles = [xpool.tile([P, KT, TS], BF16, tag="x", name=f"x{t}")
                   for t in range(TB)]
        b_tiles = {}

        def emit_b_dma(ot, eng):
            tiles = []
            for b in range(NB):
                b_sb = bpool.tile([P, G, A], FP8)
                eng.dma_start(b_sb, b_d[ot * P:(ot + 1) * P, b])
                tiles.append(b_sb)
            b_tiles[ot] = tiles

        # --- Input DMA issue.  ACT queue: alpha, b0, bias, b1 (in front of
        # its copies); Pool queue: b2, b3 (in front of its dequant muls);
        # SP queue: all x tiles.  Transfers serialize on the shared DMA
        # resource (~360 GB/s) in trigger-arrival order, so this puts the
        # dequant-chain feeds ahead of the bulk x stream without delaying it.
        nc.scalar.dma_start(al_sb, al_d.rearrange("(ot p) g nb -> p ot (g nb)", p=P))
        emit_b_dma(0, nc.scalar)
        nc.scalar.dma_start(bias_sb, bias_d.rearrange("ot p -> p ot"))
        emit_b_dma(1, nc.scalar)
        emit_b_dma(2, nc.gpsimd)
        emit_b_dma(3, nc.gpsimd)
        for t in range(TB):
            ksz = KT // x_split
            for s in range(x_split):
                ksl = slice(s * ksz, (s + 1) * ksz)
                inst = nc.sync.dma_start(x_tiles[t][:, ksl], xt_d[t, :, ksl])
                nc._x_dma[t].append(inst.ins.name)

        # --- Availability model (overridden by measured values from
        # tune.py when provided).
        if x_av_cal is not None:
            x_av = list(x_av_cal)
        else:
            x_av = [10.6 + 5.9 * t for t in range(TB)]
        if d_av_cal is not None:
            d_av = list(d_av_cal)
        else:
            d_av = [15.0 + 14.0 * ot for ot in range(OT)]

        def emit_dequant(ot):
            # W[o_p, g, a] = sum_b alpha[o_p, g, b] * binary[o_p, g, a, b]
            # accumulated in bf16 (chunked by gh groups).  Steady state:
            # bit-plane 2 is scaled on gpsimd in parallel with DVE doing
            # planes 0/1.  For o-tile 0 (head latency) the last chunk is
            # done entirely on gpsimd while DVE does the first chunks.
            bt = b_tiles[ot]
            w_sb = wpool.tile([P, G, A], BF16, tag="w", name=f"w{ot}")
            for c in range(NCH):
                gsl = slice(c * gh, (c + 1) * gh)
                t_sb = wpool.tile([P, gh, A], BF16, tag="t", name=f"t{ot}_{c}")
                t2_sb = wpool.tile([P, gh, A], BF16, tag="t2",
                                   name=f"t2_{ot}_{c}")

                def al_bc(b):
                    return al3[:, ot, gsl, b:b + 1].to_broadcast([P, gh, A])

                if ot == 0 and pool_assist and c == NCH - 1:
                    # whole chunk on gpsimd
                    i1 = nc.gpsimd.tensor_tensor(w_sb[:, gsl], bt[0][:, gsl],
                                                 al_bc(0), mult)
                    nc.gpsimd.tensor_tensor(t_sb, bt[1][:, gsl], al_bc(1), mult)
                    nc.gpsimd.tensor_tensor(w_sb[:, gsl], w_sb[:, gsl], t_sb, add)
                    nc.gpsimd.tensor_tensor(t2_sb, bt[2][:, gsl], al_bc(2), mult)
                    i2 = nc.gpsimd.tensor_tensor(w_sb[:, gsl], w_sb[:, gsl],
                                                 t2_sb, add)
                    nc._deq[ot] += [i1.ins.name, i2.ins.name]
                    continue
                if pool_assist:
                    nc.gpsimd.tensor_tensor(t2_sb, bt[2][:, gsl], al_bc(2), mult)
                i1 = nc.vector.tensor_tensor(w_sb[:, gsl], bt[0][:, gsl],
                                             al_bc(0), mult)
                nc.vector.tensor_tensor(t_sb, bt[1][:, gsl], al_bc(1), mult)
                nc.vector.tensor_tensor(w_sb[:, gsl], w_sb[:, gsl], t_sb, add)
                if not pool_assist:
                    nc.vector.tensor_tensor(t2_sb, bt[2][:, gsl], al_bc(2), mult)
                i2 = nc.vector.tensor_tensor(w_sb[:, gsl], w_sb[:, gsl],
                                             t2_sb, add)
                nc._deq[ot] += [i1.ins.name, i2.ins.name]
            return w_sb

        def emit_transpose_chunk(w_sb, wt_sb, c):
            # bf16 PE transposes (one per group) for chunk c into
            # Wt[a_p, g, o]; PSUM -> SBUF via ACT Identity copy.
            for g in range(c * gh, (c + 1) * gh):
                ps = ps_tr.tile([P, P], BF16)
                nc.tensor.matmul(ps, w_sb[:, g], identb, is_transpose=True,
                                 start=True, stop=True)
                nc.scalar.add(wt_sb[:, g, :], ps, 0.0)

        def emit_mm(wt_sb, ot, t, last=False):
            # out[o_p, t] = sum_k Wt[:, k, o]^T @ x[:, k, t]  (bf16, W-stationary)
            ps = ps_mm.tile([P, TS], F32)
            for k in range(KT):
                nc.tensor.matmul(ps, wt_sb[:, k, :], x_tiles[t][:, k, :],
                                 start=(k == 0), stop=(k == KT - 1))
            out_sb = opool.tile([P, TS], ODT)
            # PSUM -> SBUF with per-partition bias on the ACT engine.
            nc.scalar.add(out_sb, ps, bias_sb[:, ot:ot + 1])
            # the final output block goes through ACT's own DMA queue to
            # shorten the drain tail (no Pool wake-up in the chain)
            eng = nc.scalar if last else nc.gpsimd
            eng.dma_start(out_d[ot, :, t * TS:(t + 1) * TS], out_sb)

        # --- Cell schedule: emit transpose chunks and (ot, t) cells in
        # estimated-availability order so the in-order PE stream matches
        # the DMA/dequant feeds; predicted stalls are filled with junk
        # matmuls to keep the clock ramp warm.
        cell_us = KT * TS * 0.4167 / 1000 + 0.01
        tr_us = gh * (P * 0.4167 + 2.2) / 1000
        chunk_dt = gh * 5 * A * 1.0417 * 0.7 / 1000   # DVE time per chunk

        # DVE stream: all dequants up-front, in order.
        w_tiles = [emit_dequant(ot) for ot in range(OT)]
        wt_tiles = [None] * OT
        if zigzag:
            cells = sorted(
                ((max(d_av[ot], x_av[t]), ot, t)
                 for ot in range(OT) for t in range(TB)),
                key=lambda c: (c[0], c[1], c[2]))
        else:
            cells = [(0.0, ot, t) for ot in range(OT) for t in range(TB)]

        pe_t = 1.6
        for i, (av, ot, t) in enumerate(cells):
            if wt_tiles[ot] is None:
                wt_tiles[ot] = wtpool.tile([P, KT, P], BF16, tag="wt",
                                           name=f"wt{ot}")
                for c in range(NCH):
                    c_av = d_av[ot] - (NCH - 1 - c) * chunk_dt
                    if c_av > pe_t:
                        emit_junk(c_av - pe_t)
                        pe_t = c_av
                    emit_transpose_chunk(w_tiles[ot], wt_tiles[ot], c)
                    pe_t += tr_us
            if av > pe_t:
                emit_junk(av - pe_t)
                pe_t = av
            emit_mm(wt_tiles[ot], ot, t, last=(i == len(cells) - 1))
            pe_t += cell_us

    _legalize_waits(nc)
    return nc


def _stage_inputs(input, binary, alpha, bias, tb):
    TB, TS = tb, MS // tb
    bf16 = mybir.dt.np(BF16)
    fp8 = mybir.dt.np(FP8)
    x = np.ascontiguousarray(np.asarray(input, dtype=np.float32)).reshape(MS, I)
    # xt[t, p, k, s] = x[t*TS + s, k*128 + p]
    xt = np.ascontiguousarray(
        x.reshape(TB, TS, KT, P).transpose(0, 3, 2, 1)).astype(bf16)
    # binary is strictly +-1, exactly representable in fp8e4 — lossless cast.
    bperm = np.ascontiguousarray(
        np.asarray(binary, dtype=np.float32).transpose(0, 3, 1, 2)).astype(fp8)
    alpha = np.ascontiguousarray(np.asarray(alpha, dtype=np.float32))
    bias = np.asarray(bias, dtype=np.float32)

    in_maps = []
    for c in range(N_CORES):
        sl = slice(c * O_SH, (c + 1) * O_SH)
        in_maps.append({
            "xt": xt,
            "bperm": np.ascontiguousarray(bperm[sl]),
            "alpha": np.ascontiguousarray(alpha[sl]),
            "biassh": np.ascontiguousarray(bias[sl].reshape(OT, P)),
        })
    return in_maps


def kernel(input, binary, alpha, bias, _trace=False, **opts):
    key = tuple(sorted(opts.items()))
    if key not in _CACHED:
        _CACHED[key] = build_nc(**opts)
    nc = _CACHED[key]
    tb = opts.get("tb", 8)
    in_maps = _stage_inputs(input, binary, alpha, bias, tb)
    res = run_bass_kernel_spmd(nc, in_maps, core_ids=list(range(N_CORES)),
                               trace=_trace)
    # out core shard [OT, P, MS] -> [MS, O_SH]
    shards = [np.asarray(res.results[c]["out"], dtype=np.float32)
              .transpose(2, 0, 1).reshape(MS, O_SH)
              for c in range(N_CORES)]
    out = np.concatenate(shards, axis=1).reshape(B, S, O).astype(np.float32)
    kernel.last_result = res
    return out


# revision 29
# speedup vs baseline: 1.1491x; 1.1491x over previous
"""BCQLinear packed forward on 8 Trainium2 NeuronCores.

Column-parallel (tensor-parallel) sharding: binary/alpha/bias are sharded
along out_features (dim 0, 4096 -> 8 x 512); the input activations are
replicated. Each core dequantizes its weight shard
    W[o, i] = sum_b alpha[o, g, b] * binary[o, g, a, b],   i = 128 g + a
on the vector engine (gpsimd-assisted) in bf16, transposes each 128x128
group block through the PE array (one bf16 transpose per group), and runs
W-stationary bf16 matmuls  out[o, t] = Wt[:, o]^T @ x[:, t]  over token
blocks. Cells (o-tile, token-block) are emitted in DMA/dequant arrival
order so the PE never starves during the fill phase; bias is added on the
Activation engine during the PSUM->SBUF copy (per-partition bias, since
out is [o_p, t]). The host concatenates the 8 output shards along o.

Shapes are hardcoded for the problem instance:
  input  [2, 1024, 4096] f32
  binary [4096, 32, 128, 3] f32 (+-1)
  alpha  [4096, 32, 3] f32
  bias   [4096] f32
"""

import numpy as np
from contextlib import ExitStack

import bass_rust
import concourse.bass as bass
import concourse.mybir as mybir
import concourse.tile as tile
from concourse.bass_utils import run_bass_kernel_spmd
from concourse.masks import make_identity


def _legalize_waits(nc, max_waits=1):
    """Walrus codegen allows only one sync-wait on (at least) DVE
    TensorTensor instructions. Move excess waits onto injected same-engine
    NoOps placed immediately before the instruction (program order per
    engine preserves the semantics)."""
    seq = 0
    for fn in nc.m.functions:
        for blk in fn.blocks:
            new_insts = []
            changed = False
            for inst in blk.instructions:
                si = inst.sync_info
                if si is not None and len(si.on_wait) > max_waits:
                    waits = list(si.on_wait)
                    for w in waits[:-max_waits]:
                        nop = mybir.InstNoOp(name=f"wlegal-{seq}")
                        seq += 1
                        nop.engine = inst.engine
                        nop.sync_info = bass_rust.SyncInfo(
                            on_wait=[w], on_update=[])
                        new_insts.append(nop)
                    inst.sync_info = bass_rust.SyncInfo(
                        on_wait=waits[-max_waits:],
                        on_update=list(si.on_update))
                    changed = True
                new_insts.append(inst)
            if changed:
                blk.instructions = new_insts

P = 128          # partitions
N_CORES = 8
B, S = 2, 1024
MS = B * S       # 2048 tokens
I = 4096         # in_features
O = 4096         # out_features
O_SH = O // N_CORES  # 512 per core
G, A, NB = 32, 128, 3
KT = I // P      # 32 contraction tiles (== G since A == P)
OT = O_SH // P   # 4 o-tiles per core

F32 = mybir.dt.float32
BF16 = mybir.dt.bfloat16
FP8 = mybir.dt.float8e4

_CACHED = {}


def build_nc(gh: int = 8, x_split: int = 2, tb: int = 8, pool_assist: int = 1,
             dummies: int = 30, out_bf16: int = 1, zigzag: int = 1,
             x_av_cal=None, d_av_cal=None) -> bass.Bass:
    TB = tb
    TS = MS // TB
    ODT = BF16 if out_bf16 else F32
    NCH = G // gh            # dequant chunks per o-tile

    nc = bass.Bass("TRN2", target_bir_lowering=False, debug=False)

    # Host-staged layouts (pure relayouts / dtype casts of sharded inputs):
    #  xt    [TB, P, KT, TS] bf16 : xt[tb,p,k,t] = x[tb*TS+t, k*128+p]
    #  bperm [O_SH, NB, G, A] fp8 : binary shard, bit axis outward (+-1 exact)
    #  alpha [O_SH, G, NB] f32
    #  biassh[OT, P] f32          : bias shard split into o-tiles
    xt_d = nc.dram_tensor("xt", [TB, P, KT, TS], BF16, kind="ExternalInput").ap()
    b_d = nc.dram_tensor("bperm", [O_SH, NB, G, A], FP8, kind="ExternalInput").ap()
    al_d = nc.dram_tensor("alpha", [O_SH, G, NB], BF16, kind="ExternalInput").ap()
    bias_d = nc.dram_tensor("biassh", [OT, P], F32, kind="ExternalInput").ap()
    out_d = nc.dram_tensor("out", [OT, P, MS], ODT, kind="ExternalOutput").ap()

    mult = mybir.AluOpType.mult
    add = mybir.AluOpType.add

    # instruction-name maps for the calibration harness (tune.py)
    nc._x_dma = {t: [] for t in range(TB)}
    nc._deq = {ot: [] for ot in range(OT)}

    with tile.TileContext(nc) as tc, ExitStack() as ctx:
        const = ctx.enter_context(tc.tile_pool(name="const", bufs=1))
        xpool = ctx.enter_context(tc.tile_pool(name="x", bufs=TB))
        wtpool = ctx.enter_context(tc.tile_pool(name="wt", bufs=3))
        bpool = ctx.enter_context(tc.tile_pool(name="bin", bufs=6))
        wpool = ctx.enter_context(tc.tile_pool(name="w", bufs=2))
        opool = ctx.enter_context(tc.tile_pool(name="o", bufs=2))
        ps_mm = ctx.enter_context(tc.tile_pool(name="psmm", bufs=3, space="PSUM"))
        ps_tr = ctx.enter_context(tc.tile_pool(name="pstr", bufs=4, space="PSUM"))

        identb = const.tile([P, P], BF16)
        make_identity(nc, identb)
        al_sb = const.tile([P, OT, G * NB], BF16)
        bias_sb = const.tile([P, OT], F32)
        al3 = al_sb.rearrange("p ot (g nb) -> p ot g nb", nb=NB)

        # PE p-state warm-up / gap filler: junk transposes with no data
        # deps keep the PE continuously busy (the cost model restarts the
        # clock ramp at 0.65/1.2 GHz after every idle gap).
        ps_junk = ps_tr.tile([P, P], BF16, tag="junk", bufs=1)

        def emit_junk(us):
            for _ in range(min(400, int(us / 0.0533))):
                nc.tensor.matmul(ps_junk, identb, identb, is_transpose=True,
                                 start=True, stop=True)

        x_tiles = [xpool.tile([P, KT, TS], BF16, tag="x", name=f"x{t}")
                   for t in range(TB)]
        b_tiles = {}

        def emit_b_dma(ot, eng):
            tiles = []
            for b in range(NB):
                b_sb = bpool.tile([P, G, A], FP8)
                eng.dma_start(b_sb, b_d[ot * P:(ot + 1) * P, b])
                tiles.append(b_sb)
            b_tiles[ot] = tiles

        # --- Input DMA issue.  b0 goes out first on the SP queue (it heads
        # the dequant chain and must beat the x stream to the wire); alpha
        # and b1 ride the ACT queue; b2/b3 are triggered from the Pool
        # queue at deferred positions inside the dequant stream (see
        # emit_dequant) so their buffer-ring waits cannot block Pool work
        # and their transfers do not steal early wire slots from x.
        # Transfers serialize on the shared DMA resource (~360 GB/s) in
        # trigger-arrival order.
        nc.scalar.dma_start(al_sb, al_d.rearrange("(ot p) g nb -> p ot (g nb)", p=P))
        nc.scalar.dma_start(bias_sb, bias_d.rearrange("ot p -> p ot"))

        def emit_x_dma(t):
            ksz = KT // x_split
            for s in range(x_split):
                ksl = slice(s * ksz, (s + 1) * ksz)
                inst = nc.sync.dma_start(x_tiles[t][:, ksl], xt_d[t, :, ksl])
                nc._x_dma[t].append(inst.ins.name)

        # SP wire order: b(ot) interleaves with x so each lands about one
        # dequant-latency ahead of its PE phase. With 11 ring slots only
        # b3's last plane reuses a buffer (b0's first, long freed).
        emit_b_dma(0, nc.sync)
        emit_x_dma(0)
        emit_b_dma(1, nc.sync)
        emit_x_dma(1)
        emit_x_dma(2)
        emit_b_dma(2, nc.sync)
        emit_x_dma(3)
        emit_x_dma(4)
        emit_b_dma(3, nc.sync)
        for t in range(5, TB):
            emit_x_dma(t)

        # --- Availability model (overridden by measured values from
        # tune.py when provided).
        if x_av_cal is not None:
            x_av = list(x_av_cal)
        else:
            x_av = [14.2, 24.39, 30.22, 41.68, 47.69, 60.42,
                    66.43, 73.71][:TB]
        if d_av_cal is not None:
            d_av = list(d_av_cal)
        else:
            d_av = [22.87, 33.93, 48.26, 63.68][:OT]

        def emit_dequant(ot):
            # W[o_p, g, a] = sum_b alpha[o_p, g, b] * binary[o_p, g, a, b]
            # in gh-group chunks.  o-tile 0 (head latency): DVE computes
            # w = a0*B0 + a1*B1, gpsimd computes t2 = a2*B2, and the PE
            # transpose pass accumulates w^T + t2^T — the third plane's
            # add rides the PE, so the first transpose starts ~5us after
            # b0 lands.  o-tiles 1-3: full 5-op dequant (DVE + gpsimd m2
            # assist), single transpose per group.
            bt = b_tiles[ot]
            w_sb = wpool.tile([P, G, A], BF16, tag="w", name=f"w{ot}")
            t2cs = []
            for c in range(NCH):
                gsl = slice(c * gh, (c + 1) * gh)
                t_sb = wpool.tile([P, gh, A], BF16, tag="t",
                                  name=f"t{ot}_{c}", bufs=1)
                t2_sb = wpool.tile([P, gh, A], BF16, tag="t2",
                                   name=f"t2_{ot}_{c}", bufs=2)
                t2cs.append(t2_sb)

                def al_bc(b):
                    return al3[:, ot, gsl, b:b + 1].to_broadcast([P, gh, A])

                if pool_assist:
                    i1 = nc.gpsimd.tensor_tensor(t2_sb, bt[2][:, gsl],
                                                 al_bc(2), mult)
                nc.vector.tensor_tensor(w_sb[:, gsl], bt[0][:, gsl], al_bc(0), mult)
                nc.vector.tensor_tensor(t_sb, bt[1][:, gsl], al_bc(1), mult)
                i2 = nc.vector.tensor_tensor(w_sb[:, gsl], w_sb[:, gsl], t_sb, add)
                if not pool_assist:
                    i1 = nc.vector.tensor_tensor(t2_sb, bt[2][:, gsl],
                                                 al_bc(2), mult)
                i2 = nc.vector.tensor_tensor(w_sb[:, gsl], w_sb[:, gsl],
                                             t2_sb, add)
                nc._deq[ot] += [i1.ins.name, i2.ins.name]
            return w_sb, t2cs

        def emit_transpose_chunk(w_parts, wt_sb, c, fused):
            # bf16 PE transposes per group block into Wt[a_p, g, o]
            # (PSUM -> SBUF via ACT Identity copy).  In fused mode the
            # third plane is accumulated through a second transpose.
            w_sb, t2cs = w_parts
            for g in range(c * gh, (c + 1) * gh):
                ps = ps_tr.tile([P, P], BF16)
                nc.tensor.matmul(ps, w_sb[:, g], identb, is_transpose=True,
                                 start=True, stop=True)
                nc.scalar.add(wt_sb[:, g, :], ps, 0.0)

        n_out = [0]

        def emit_mm(wt_sb, ot, t, last=False):
            # out[o_p, t] = sum_k Wt[:, k, o]^T @ x[:, k, t]  (bf16, W-stationary)
            ps = ps_mm.tile([P, TS], F32)
            for k in range(KT):
                nc.tensor.matmul(ps, wt_sb[:, k, :], x_tiles[t][:, k, :],
                                 start=(k == 0), stop=(k == KT - 1))
            out_sb = opool.tile([P, TS], ODT)
            # PSUM -> SBUF with per-partition bias on the ACT engine.  The
            # first cells drain through ACT's own DMA queue (prompt, keeps
            # the small out ring turning); later cells drain through SP,
            # whose triggers queue behind all input DMAs so their wire
            # slots defer past the x stream.  Pool stays dequant-only.
            nc.scalar.add(out_sb, ps, bias_sb[:, ot:ot + 1])
            eng = nc.scalar if (n_out[0] < 12 or last) else nc.sync
            n_out[0] += 1
            eng.dma_start(out_d[ot, :, t * TS:(t + 1) * TS], out_sb)

        # --- Cell schedule: emit transpose chunks and (ot, t) cells in
        # estimated-availability order so the in-order PE stream matches
        # the DMA/dequant feeds; predicted stalls are filled with junk
        # matmuls to keep the clock ramp warm.
        cell_us = KT * TS * 0.4167 / 1000 + 0.01
        tr_us = gh * (P * 0.4167 + 2.2) / 1000
        chunk_dt = gh * A * 1.0417 * 4.0 / 1000   # DVE time per chunk

        # DVE stream: all dequants up-front, in order.
        w_tiles = [emit_dequant(ot) for ot in range(OT)]
        wt_tiles = [None] * OT
        if zigzag:
            cells = sorted(
                ((max(d_av[ot], x_av[t]), ot, t)
                 for ot in range(OT) for t in range(TB)),
                key=lambda c: (c[0], c[1], c[2]))
        else:
            cells = [(0.0, ot, t) for ot in range(OT) for t in range(TB)]

        # wt ring is 3 deep: before opening o-tile N, flush any remaining
        # cells of o-tile N-3 (their wt buffer is about to be recycled).
        order = []
        opened = []
        pending = {o: [c for c in cells if c[1] == o] for o in range(OT)}
        for c in cells:
            if c not in pending[c[1]]:
                continue
            if c[1] not in opened:
                opened.append(c[1])
                if len(opened) > 3:
                    old_ot = opened[-4]
                    order += pending[old_ot]
                    pending[old_ot] = []
            pending[c[1]].remove(c)
            order.append(c)
        cells = order

        pe_t = 1.6
        for i, (av, ot, t) in enumerate(cells):
            if wt_tiles[ot] is None:
                wt_tiles[ot] = wtpool.tile([P, KT, P], BF16, tag="wt",
                                           name=f"wt{ot}")
                for c in range(NCH):
                    c_av = d_av[ot] - (NCH - 1 - c) * chunk_dt
                    if c_av > pe_t:
                        emit_junk(c_av - pe_t)
                        pe_t = c_av
                    emit_transpose_chunk(w_tiles[ot], wt_tiles[ot], c,
                                         fused=False)
                    pe_t += tr_us
            if av > pe_t:
                emit_junk(av - pe_t)
                pe_t = av
            emit_mm(wt_tiles[ot], ot, t, last=(i == len(cells) - 1))
            pe_t += cell_us

    _legalize_waits(nc)
    return nc


def _stage_inputs(input, binary, alpha, bias, tb):
    TB, TS = tb, MS // tb
    bf16 = mybir.dt.np(BF16)
    fp8 = mybir.dt.np(FP8)
    x = np.ascontiguousarray(np.asarray(input, dtype=np.float32)).reshape(MS, I)
    # xt[t, p, k, s] = x[t*TS + s, k*128 + p]
    xt = np.ascontiguousarray(
        x.reshape(TB, TS, KT, P).transpose(0, 3, 2, 1)).astype(bf16)
    # binary is strictly +-1, exactly representable in fp8e4 — lossless cast.
    bperm = np.ascontiguousarray(
        np.asarray(binary, dtype=np.float32).transpose(0, 3, 1, 2)).astype(fp8)
    alpha = np.ascontiguousarray(np.asarray(alpha, dtype=np.float32))
    bias = np.asarray(bias, dtype=np.float32)

    in_maps = []
    for c in range(N_CORES):
        sl = slice(c * O_SH, (c + 1) * O_SH)
        in_maps.append({
            "xt": xt,
            "bperm": np.ascontiguousarray(bperm[sl]),
            "alpha": np.ascontiguousarray(alpha[sl]).astype(bf16),
            "biassh": np.ascontiguousarray(bias[sl].reshape(OT, P)),
        })
    return in_maps


def kernel(input, binary, alpha, bias, _trace=False, **opts):
    key = tuple(sorted(opts.items()))
    if key not in _CACHED:
        _CACHED[key] = build_nc(**opts)
    nc = _CACHED[key]
    tb = opts.get("tb", 8)
    in_maps = _stage_inputs(input, binary, alpha, bias, tb)
    res = run_bass_kernel_spmd(nc, in_maps, core_ids=list(range(N_CORES)),
                               trace=_trace)
    # out core shard [OT, P, MS] -> [MS, O_SH]
    shards = [np.asarray(res.results[c]["out"], dtype=np.float32)
              .transpose(2, 0, 1).reshape(MS, O_SH)
              for c in range(N_CORES)]
    out = np.concatenate(shards, axis=1).reshape(B, S, O).astype(np.float32)
    kernel.last_result = res
    return out
